# revision 10
# baseline (speedup 1.0000x reference)
"""Trainium2 Bass kernel for the allpass-delay LSTM scan (nn_APDL_RNN).

Reference computation per time step t (B=16, T=16384, I=1, H=64):
    ap_t = coeff*(s_{t-1} - ap_{t-1}) + s_{t-2}        (allpass state, [B, 2H])
    gates = x_t @ W_ih.T + b_ih + ap_t[:, :H] @ W_hh.T + b_hh
    i,f,g,o = split(gates); c_t = sig(f)*ap_t[:, H:] + sig(i)*tanh(g)
    h_t = sig(o)*tanh(c_t); s_t = [h_t, c_t]
Outputs: y = h states [B,T,H], c states [B,T,H], ap_final [B,2H].

Sharding: data-parallel over batch, 2 rows per NeuronCore, weights
replicated. Each core runs the full sequential scan on its shard.

On-chip layout: features on partitions, the core's 2 batch rows in the
free dim at column offsets {s, R+s} (block layout, R = steps/chunk).
Per step: PE does gates = [x;1] @ Wx (K=2) + ap_h @ Wh (K=64) into two
PSUM tiles (i|f and g|o); ACT applies sigmoid/tanh; DVE does the cell
products and the allpass recurrence via scalar_tensor_tensor.
A Fori loop runs NCHUNK chunks of R fully-unrolled steps with static
APs; chunk boundaries DMA x in / states out, clear semaphores, barrier.
"""

import sys

if "/opt/trn_rl_repo" not in sys.path:
    sys.path.insert(0, "/opt/trn_rl_repo")

import math

import numpy as np

import concourse.bass as bass
import concourse.mybir as mybir
from concourse.bass import ds
from concourse.bass_utils import run_bass_kernel_spmd

B, T, I, H = 16, 16384, 1, 64
OS = 48000.0 / 44100.0
ALPHA = OS - math.floor(OS)
COEFF = (1.0 - ALPHA) / (1.0 + ALPHA)

NCORE = 8
BS = B // NCORE  # 2 batch rows per core

DT = mybir.dt.float32
AF = mybir.ActivationFunctionType
ALU = mybir.AluOpType


def build_nc(T_, R_, drains=False):
    """Build the SPMD Bass program for one core (2 batch rows, T_ steps)."""
    assert T_ % R_ == 0
    nchunk = T_ // R_
    R2 = 2 * R_

    nc = bass.Bass(target_bir_lowering=False)

    x_d = nc.dram_tensor("x", [2, T_ + R_], DT, kind="ExternalInput")
    w_x = nc.dram_tensor("wx", [2, 256], DT, kind="ExternalInput")
    w_h = nc.dram_tensor("wh", [64, 256], DT, kind="ExternalInput")
    yh = nc.dram_tensor("yh", [2, 64, T_], DT, kind="ExternalOutput")
    cs = nc.dram_tensor("cs", [2, 64, T_], DT, kind="ExternalOutput")
    apf = nc.dram_tensor("apf", [128, 2], DT, kind="ExternalOutput")

    from contextlib import ExitStack

    with ExitStack() as ctx:
        WX = ctx.enter_context(nc.sbuf_tensor("WX", [2, 256], DT))
        WH = ctx.enter_context(nc.sbuf_tensor("WH", [64, 256], DT))
        XS = ctx.enter_context(nc.sbuf_tensor("XS", [2, R2], DT))
        APR = ctx.enter_context(nc.sbuf_tensor("APR", [128, R2], DT))
        SC = ctx.enter_context(nc.sbuf_tensor("SC", [128, R2], DT))
        ER = ctx.enter_context(nc.sbuf_tensor("ER", [128, R2], DT))
        SIG = ctx.enter_context(nc.sbuf_tensor("SIG", [128, R2], DT))
        GG = ctx.enter_context(nc.sbuf_tensor("GG", [64, R2], DT))
        OG = ctx.enter_context(nc.sbuf_tensor("OG", [64, R2], DT))
        TH = ctx.enter_context(nc.sbuf_tensor("TH", [64, R2], DT))
        T1 = ctx.enter_context(nc.sbuf_tensor("T1", [64, R2], DT))
        T2 = ctx.enter_context(nc.sbuf_tensor("T2", [64, R2], DT))
        psA0 = ctx.enter_context(nc.psum_tensor("psA0", [128, 2], DT))
        psB0 = ctx.enter_context(nc.psum_tensor("psB0", [128, 2], DT))
        psA1 = ctx.enter_context(nc.psum_tensor("psA1", [128, 2], DT))
        psB1 = ctx.enter_context(nc.psum_tensor("psB1", [128, 2], DT))
        s_ap = ctx.enter_context(nc.semaphore("s_ap"))
        s_mm = ctx.enter_context(nc.semaphore("s_mm"))
        s_act = ctx.enter_context(nc.semaphore("s_act"))
        s_cn = ctx.enter_context(nc.semaphore("s_cn"))
        s_th = ctx.enter_context(nc.semaphore("s_th"))
        dsem = ctx.enter_context(nc.semaphore("dsem"))
        psA = [psA0, psA1]
        psB = [psB0, psB1]
        g = nc.gpsimd

        def cols(t, s):
            # both batch rows' columns for ring slot s
            return t[:, s:R2:R_]

        # ---- preamble: weights, first x chunk, zero initial state ----
        g.dma_start(WX[:, :], w_x[:, :]).then_inc(dsem, 16)
        g.dma_start(WH[:, :], w_h[:, :]).then_inc(dsem, 16)
        g.dma_start(XS[1:2, 0:R_], x_d[0:1, 0:R_]).then_inc(dsem, 16)
        g.dma_start(XS[1:2, R_:R2], x_d[1:2, 0:R_]).then_inc(dsem, 16)
        g.memset(XS[0:1, :], 1.0)
        g.memset(APR[:, 0:R2:R_], 0.0)  # ap_0 = 0
        g.memset(APR[:, R_ - 1 : R2 : R_], 0.0)  # ap_{-1} slot
        g.memset(SC[:, R_ - 1 : R2 : R_], 0.0)  # state_{-1} = 0
        g.memset(SC[:, R_ - 2 : R2 : R_], 0.0)  # state_{-2} = 0
        g.wait_ge(dsem, 64)
        nc.all_engine_barrier()

        with nc.Fori(0, nchunk) as i:
            # ---------------- PE ----------------
            pe = nc.tensor
            for s in range(R_):
                pa, pb = psA[s % 2], psB[s % 2]
                xm = XS[:, s:R2:R_]
                if s >= 2:
                    pe.wait_ge(s_act, s - 1)
                pe.matmul(pa[:, :], WX[:, 0:128], xm, start=True, stop=False)
                pe.matmul(pb[:, :], WX[:, 128:256], xm, start=True, stop=False)
                if s >= 1:
                    pe.wait_ge(s_ap, s)
                am = APR[0:64, s:R2:R_]
                pe.matmul(pa[:, :], WH[:, 0:128], am, start=False, stop=True)
                pe.matmul(pb[:, :], WH[:, 128:256], am, start=False, stop=True).then_inc(s_mm, 1)

            # ---------------- ACT ----------------
            a = nc.scalar
            for s in range(R_):
                pa, pb = psA[s % 2], psB[s % 2]
                a.wait_ge(s_mm, s + 1)
                a.activation(cols(SIG, s), pa[:, :], AF.Sigmoid)
                a.activation(cols(GG, s), pb[0:64, :], AF.Tanh)
                a.activation(cols(OG, s), pb[64:128, :], AF.Sigmoid).then_inc(s_act, 1)
                a.wait_ge(s_cn, s + 1)
                a.activation(cols(TH, s), SC[64:128, s:R2:R_], AF.Tanh).then_inc(s_th, 1)

            # ---------------- DVE ----------------
            v = nc.vector
            for s in range(R_):
                sp = (s - 1) % R_  # slot s-1
                sn = (s + 1) % R_  # slot s+1
                if drains:
                    v.drain()
                # e_{s+1} = -coeff*ap_s + state_{s-1}
                v.scalar_tensor_tensor(
                    cols(ER, sn), cols(APR, s), -COEFF, cols(SC, sp),
                    op0=ALU.mult, op1=ALU.add,
                )
                v.wait_ge(s_act, s + 1)
                v.tensor_tensor(cols(T1, s), SIG[0:64, s:R2:R_], cols(GG, s), op=ALU.mult)
                v.tensor_tensor(
                    cols(T2, s), SIG[64:128, s:R2:R_],
                    APR[64:128, s:R2:R_], op=ALU.mult,
                )
                if drains:
                    v.drain()
                v.tensor_tensor(
                    SC[64:128, s:R2:R_], cols(T1, s), cols(T2, s), op=ALU.add
                ).then_inc(s_cn, 1)
                v.wait_ge(s_th, s + 1)
                v.tensor_tensor(SC[0:64, s:R2:R_], cols(OG, s), cols(TH, s), op=ALU.mult)
                if drains:
                    v.drain()
                # ap_{s+1} = coeff*state_s + e_{s+1}
                v.scalar_tensor_tensor(
                    cols(APR, sn), cols(SC, s), COEFF, cols(ER, sn),
                    op0=ALU.mult, op1=ALU.add,
                ).then_inc(s_ap, 1)

            # ---------------- chunk boundary (gpsimd) ----------------
            g.wait_ge(s_ap, R_)
            g.dma_start(yh[0:1, :, ds(i * R_, R_)], SC[0:64, 0:R_]).then_inc(dsem, 16)
            g.dma_start(yh[1:2, :, ds(i * R_, R_)], SC[0:64, R_:R2]).then_inc(dsem, 16)
            g.dma_start(cs[0:1, :, ds(i * R_, R_)], SC[64:128, 0:R_]).then_inc(dsem, 16)
            g.dma_start(cs[1:2, :, ds(i * R_, R_)], SC[64:128, R_:R2]).then_inc(dsem, 16)
            g.dma_start(XS[1:2, 0:R_], x_d[0:1, ds((i + 1) * R_, R_)]).then_inc(dsem, 16)
            g.dma_start(XS[1:2, R_:R2], x_d[1:2, ds((i + 1) * R_, R_)]).then_inc(dsem, 16)
            g.wait_ge(dsem, 64 + 96 * (i + 1))
            nc.all_engine_barrier()
            for sem in (s_ap, s_mm, s_act, s_cn, s_th):
                g.sem_clear(sem)
            nc.all_engine_barrier()

        # ---- postamble: ap_final = ap_{T-1} (ring slot R-1) ----
        with nc.allow_non_contiguous_dma(reason="final 2-col allpass state"):
            g.dma_start(apf[:, :], APR[:, R_ - 1 : R2 : R_]).then_inc(dsem, 16)
        g.wait_ge(dsem, 64 + 96 * nchunk + 16)
    return nc


def _build_inputs(x, W_ih, W_hh, b_ih, b_hh, T_, R_):
    wx = np.empty((2, 256), np.float32)
    wx[0] = b_ih + b_hh
    wx[1] = W_ih[:, 0]
    wh = np.ascontiguousarray(W_hh.T, dtype=np.float32)  # [64, 256]
    in_maps = []
    for c in range(NCORE):
        xp = np.zeros((2, T_ + R_), np.float32)
        xp[:, :T_] = x[2 * c : 2 * c + 2, :, 0]
        in_maps.append({"x": xp, "wx": wx, "wh": wh})
    return in_maps


R_CHUNK = 512
DRAINS = True
_NC_CACHE = {}


def _get_nc(T_, R_, drains):
    key = (T_, R_, drains)
    if key not in _NC_CACHE:
        _NC_CACHE[key] = build_nc(T_, R_, drains=drains)
    return _NC_CACHE[key]


def kernel(x, W_ih, W_hh, b_ih, b_hh):
    x = np.asarray(x, np.float32)
    W_ih = np.asarray(W_ih, np.float32)
    W_hh = np.asarray(W_hh, np.float32)
    b_ih = np.asarray(b_ih, np.float32)
    b_hh = np.asarray(b_hh, np.float32)

    R_ = R_CHUNK
    nc = _get_nc(T, R_, DRAINS)
    in_maps = _build_inputs(x, W_ih, W_hh, b_ih, b_hh, T, R_)
    res = run_bass_kernel_spmd(nc, in_maps, core_ids=list(range(NCORE)))

    y = np.empty((B, T, H), np.float32)
    c_out = np.empty((B, T, H), np.float32)
    ap_final = np.empty((B, 2 * H), np.float32)
    for c in range(NCORE):
        r = res.results[c]
        for b in range(2):
            y[2 * c + b] = r["yh"][b].T
            c_out[2 * c + b] = r["cs"][b].T
            ap_final[2 * c + b] = r["apf"][:, b]
    return y, c_out, ap_final


# revision 22
# speedup vs baseline: 1.2445x; 1.2445x over previous
"""Trainium2 Bass kernel for the allpass-delay LSTM scan (nn_APDL_RNN).

Reference computation per time step t (B=16, T=16384, I=1, H=64):
    ap_t = coeff*(s_{t-1} - ap_{t-1}) + s_{t-2}        (allpass state, [B, 2H])
    gates = x_t @ W_ih.T + b_ih + ap_t[:, :H] @ W_hh.T + b_hh
    i,f,g,o = split(gates); c_t = sig(f)*ap_t[:, H:] + sig(i)*tanh(g)
    h_t = sig(o)*tanh(c_t); s_t = [h_t, c_t]
Outputs: y = h states [B,T,H], c states [B,T,H], ap_final [B,2H].

Sharding: data-parallel over batch, 2 rows per NeuronCore, weights
replicated. Each core runs the full sequential scan on its shard.

On-chip layout: features on partitions, the core's 2 batch rows in the
free dim at column offsets {s, R+s} (block layout, R = steps/chunk).
Per step: PE does gates = [x;1] @ Wx (K=2) + ap_h @ Wh (K=64) into two
PSUM tiles (i|f and g|o); ACT applies sigmoid/tanh; DVE does the cell
products and the allpass recurrence via scalar_tensor_tensor.
A Fori loop runs NCHUNK chunks of R fully-unrolled steps with static
APs. All semaphores count monotonically across chunks (thresholds are
per-engine counter registers); chunk boundaries just issue DMAs (x in,
states out) — no barriers or sem_clears in the loop, both cost ~ms on
HW. DVE same-engine RAW hazards are handled by instruction ordering
(every dependent pair has >=1 intervening DVE op; the DVE pipeline does
not interlock, so back-to-back dependent ops read stale data).
"""

import sys

if "/opt/trn_rl_repo" not in sys.path:
    sys.path.insert(0, "/opt/trn_rl_repo")

import math

import numpy as np

import concourse.bass as bass
import concourse.mybir as mybir
from concourse.bass import ds
from concourse.bass_utils import run_bass_kernel_spmd

B, T, I, H = 16, 16384, 1, 64
OS = 48000.0 / 44100.0
ALPHA = OS - math.floor(OS)
COEFF = (1.0 - ALPHA) / (1.0 + ALPHA)

NCORE = 8
BS = B // NCORE  # 2 batch rows per core

DT = mybir.dt.float32
AF = mybir.ActivationFunctionType
ALU = mybir.AluOpType


def build_nc(T_, R_, drains=False, exec_chunks=None):
    """Build the SPMD Bass program for one core (2 batch rows, T_ steps)."""
    assert T_ % R_ == 0
    nchunk = T_ // R_
    if exec_chunks is not None:
        nchunk = exec_chunks
    R2 = 2 * R_

    nc = bass.Bass(target_bir_lowering=False)

    x_d = nc.dram_tensor("x", [2, T_ + R_], DT, kind="ExternalInput")
    w_x = nc.dram_tensor("wx", [2, 256], DT, kind="ExternalInput")
    w_h = nc.dram_tensor("wh", [64, 256], DT, kind="ExternalInput")
    yh = nc.dram_tensor("yh", [2, 64, T_], DT, kind="ExternalOutput")
    cs = nc.dram_tensor("cs", [2, 64, T_], DT, kind="ExternalOutput")
    apf = nc.dram_tensor("apf", [128, 2], DT, kind="ExternalOutput")

    from contextlib import ExitStack

    with ExitStack() as ctx:
        WX = ctx.enter_context(nc.sbuf_tensor("WX", [2, 256], DT))
        WH = ctx.enter_context(nc.sbuf_tensor("WH", [64, 256], DT))
        XS = ctx.enter_context(nc.sbuf_tensor("XS", [2, R2], DT))
        APR = ctx.enter_context(nc.sbuf_tensor("APR", [128, R2], DT))
        SC = ctx.enter_context(nc.sbuf_tensor("SC", [128, R2], DT))
        ER = ctx.enter_context(nc.sbuf_tensor("ER", [128, R2], DT))
        SIG = ctx.enter_context(nc.sbuf_tensor("SIG", [128, R2], DT))
        GG = ctx.enter_context(nc.sbuf_tensor("GG", [64, R2], DT))
        OG = ctx.enter_context(nc.sbuf_tensor("OG", [64, R2], DT))
        TH = ctx.enter_context(nc.sbuf_tensor("TH", [64, R2], DT))
        T1 = ctx.enter_context(nc.sbuf_tensor("T1", [64, R2], DT))
        T2 = ctx.enter_context(nc.sbuf_tensor("T2", [64, R2], DT))
        psA0 = ctx.enter_context(nc.psum_tensor("psA0", [128, 2], DT))
        psB0 = ctx.enter_context(nc.psum_tensor("psB0", [128, 2], DT))
        psA1 = ctx.enter_context(nc.psum_tensor("psA1", [128, 2], DT))
        psB1 = ctx.enter_context(nc.psum_tensor("psB1", [128, 2], DT))
        s_ap = ctx.enter_context(nc.semaphore("s_ap"))
        s_mm = ctx.enter_context(nc.semaphore("s_mm"))
        s_act = ctx.enter_context(nc.semaphore("s_act"))
        s_cn = ctx.enter_context(nc.semaphore("s_cn"))
        s_th = ctx.enter_context(nc.semaphore("s_th"))
        dsem = ctx.enter_context(nc.semaphore("dsem"))
        psA = [psA0, psA1]
        psB = [psB0, psB1]
        g = nc.gpsimd

        def cols(t, s):
            # both batch rows' columns for ring slot s
            return t[:, s:R2:R_]

        # ---- preamble: weights, first x chunk, zero initial state ----
        g.dma_start(WX[:, :], w_x[:, :]).then_inc(dsem, 16)
        g.dma_start(WH[:, :], w_h[:, :]).then_inc(dsem, 16)
        g.dma_start(XS[1:2, 0:R_], x_d[0:1, 0:R_]).then_inc(dsem, 16)
        g.dma_start(XS[1:2, R_:R2], x_d[1:2, 0:R_]).then_inc(dsem, 16)
        g.memset(XS[0:1, :], 1.0)
        g.memset(APR[:, 0:R2:R_], 0.0)  # ap_0 = 0
        g.memset(APR[:, R_ - 1 : R2 : R_], 0.0)  # ap_{-1} slot
        g.memset(SC[:, R_ - 1 : R2 : R_], 0.0)  # state_{-1} = 0
        g.memset(SC[:, R_ - 2 : R2 : R_], 0.0)  # state_{-2} = 0
        # s_act gets a +1 preamble offset so the PE's PSUM-WAR threshold
        # (semantically i*R + s - 1) never goes negative at i=s=0.
        g.sem_inc(s_act, 1)
        g.wait_ge(dsem, 64)
        nc.all_engine_barrier()

        # All semaphores count monotonically across chunks; thresholds are
        # register expressions of the chunk index. No barriers or clears in
        # the loop (all_engine_barrier + sem_clear cost ~ms on HW).
        rp = nc.tensor.alloc_register("rp")
        ra = nc.scalar.alloc_register("ra")
        rv1 = nc.vector.alloc_register("rv1")
        rv2 = nc.vector.alloc_register("rv2")
        with nc.Fori(0, nchunk) as i:
            base = i * R_
            # ---------------- PE ----------------
            pe = nc.tensor
            pe.wait_ge(dsem, 64 + 96 * i)  # chunk-i x staged, chunk-(i-1) flushed
            pe.reg_alu(rp, i, R_, op=ALU.mult)  # rp = i*R
            for s in range(R_):
                pa, pb = psA[s % 2], psB[s % 2]
                xm = XS[:, s:R2:R_]
                # Single wait: s_ap >= i*R+s implies (via DVE step s-1 having
                # consumed ACT's tanh_c(s-1), which follows sig_o(s-2) on the
                # in-order ACT stream) that ACT's reads of psum parity s%2 are
                # done — so no separate PSUM-WAR wait on s_act is needed.
                pe.wait_ge(s_ap, rp)
                pe.matmul(pa[:, :], WX[:, 0:128], xm, start=True, stop=False)
                pe.matmul(pb[:, :], WX[:, 128:256], xm, start=True, stop=False)
                am = APR[0:64, s:R2:R_]
                pe.matmul(pa[:, :], WH[:, 0:128], am, start=False, stop=True)
                pe.matmul(pb[:, :], WH[:, 128:256], am, start=False, stop=True).then_inc(s_mm, 1)
                pe.reg_add(rp, rp, 1)

            # ---------------- ACT ----------------
            a = nc.scalar
            a.reg_alu(ra, i, R_, op=ALU.mult)
            a.reg_add(ra, ra, 1)  # ra = i*R + 1
            for s in range(R_):
                pa, pb = psA[s % 2], psB[s % 2]
                a.wait_ge(s_mm, ra)
                a.activation(cols(SIG, s), pa[:, :], AF.Sigmoid)
                a.activation(cols(GG, s), pb[0:64, :], AF.Tanh)
                a.activation(cols(OG, s), pb[64:128, :], AF.Sigmoid).then_inc(s_act, 1)
                a.wait_ge(s_cn, ra)
                a.activation(cols(TH, s), SC[64:128, s:R2:R_], AF.Tanh).then_inc(s_th, 1)
                a.reg_add(ra, ra, 1)

            # ---------------- DVE ----------------
            v = nc.vector
            v.wait_ge(dsem, 64 + 96 * i)  # chunk-(i-1) SC flush done before overwrite
            v.reg_alu(rv1, i, R_, op=ALU.mult)
            v.reg_add(rv1, rv1, 2)  # rv1 = i*R + 2  (s_act threshold, +1 offset)
            v.reg_alu(rv2, i, R_, op=ALU.mult)
            v.reg_add(rv2, rv2, 1)  # rv2 = i*R + 1  (s_th threshold)
            for s in range(R_):
                sp = (s - 1) % R_  # slot s-1
                sn = (s + 1) % R_  # slot s+1
                if drains == "gap":
                    # RAW-safe ordering: every DVE-to-DVE dependent pair has
                    # >=1 intervening DVE instruction instead of a drain.
                    v.wait_ge(s_act, rv1)
                    v.tensor_tensor(cols(T1, s), SIG[0:64, s:R2:R_], cols(GG, s), op=ALU.mult)
                    v.tensor_tensor(cols(T2, s), SIG[64:128, s:R2:R_], APR[64:128, s:R2:R_], op=ALU.mult)
                    # e_{s+1} = -coeff*ap_s + state_{s-1} (gap filler, reads ap_s)
                    v.scalar_tensor_tensor(cols(ER, sn), cols(APR, s), -COEFF, cols(SC, sp), op0=ALU.mult, op1=ALU.add)
                    v.tensor_tensor(SC[64:128, s:R2:R_], cols(T1, s), cols(T2, s), op=ALU.add).then_inc(s_cn, 1)
                    v.wait_ge(s_th, rv2)
                    v.tensor_tensor(SC[0:64, s:R2:R_], cols(OG, s), cols(TH, s), op=ALU.mult)
                    # ap_{s+1} split h/c so each half has a gap from its input
                    v.scalar_tensor_tensor(APR[64:128, sn:R2:R_], SC[64:128, s:R2:R_], COEFF, ER[64:128, sn:R2:R_], op0=ALU.mult, op1=ALU.add)
                    v.scalar_tensor_tensor(APR[0:64, sn:R2:R_], SC[0:64, s:R2:R_], COEFF, ER[0:64, sn:R2:R_], op0=ALU.mult, op1=ALU.add).then_inc(s_ap, 1)
                    v.reg_add(rv1, rv1, 1)
                    v.reg_add(rv2, rv2, 1)
                    continue
                if drains:
                    v.drain()
                # e_{s+1} = -coeff*ap_s + state_{s-1}
                v.scalar_tensor_tensor(
                    cols(ER, sn), cols(APR, s), -COEFF, cols(SC, sp),
                    op0=ALU.mult, op1=ALU.add,
                )
                v.wait_ge(s_act, rv1)  # gates of step s ready (+1 offset)
                v.tensor_tensor(cols(T1, s), SIG[0:64, s:R2:R_], cols(GG, s), op=ALU.mult)
                v.tensor_tensor(
                    cols(T2, s), SIG[64:128, s:R2:R_],
                    APR[64:128, s:R2:R_], op=ALU.mult,
                )
                if drains:
                    v.drain()
                v.tensor_tensor(
                    SC[64:128, s:R2:R_], cols(T1, s), cols(T2, s), op=ALU.add
                ).then_inc(s_cn, 1)
                v.wait_ge(s_th, rv2)
                v.tensor_tensor(SC[0:64, s:R2:R_], cols(OG, s), cols(TH, s), op=ALU.mult)
                if drains:
                    v.drain()
                # ap_{s+1} = coeff*state_s + e_{s+1}
                v.scalar_tensor_tensor(
                    cols(APR, sn), cols(SC, s), COEFF, cols(ER, sn),
                    op0=ALU.mult, op1=ALU.add,
                ).then_inc(s_ap, 1)
                v.reg_add(rv1, rv1, 1)
                v.reg_add(rv2, rv2, 1)

            # ---------------- chunk boundary (gpsimd, no barrier) ----------------
            g.wait_ge(s_ap, base + R_)
            g.dma_start(yh[0:1, :, ds(i * R_, R_)], SC[0:64, 0:R_]).then_inc(dsem, 16)
            g.dma_start(yh[1:2, :, ds(i * R_, R_)], SC[0:64, R_:R2]).then_inc(dsem, 16)
            g.dma_start(cs[0:1, :, ds(i * R_, R_)], SC[64:128, 0:R_]).then_inc(dsem, 16)
            g.dma_start(cs[1:2, :, ds(i * R_, R_)], SC[64:128, R_:R2]).then_inc(dsem, 16)
            g.dma_start(XS[1:2, 0:R_], x_d[0:1, ds((i + 1) * R_, R_)]).then_inc(dsem, 16)
            g.dma_start(XS[1:2, R_:R2], x_d[1:2, ds((i + 1) * R_, R_)]).then_inc(dsem, 16)

        # ---- postamble: ap_final = ap_{T-1} (ring slot R-1) ----
        with nc.allow_non_contiguous_dma(reason="final 2-col allpass state"):
            g.dma_start(apf[:, :], APR[:, R_ - 1 : R2 : R_]).then_inc(dsem, 16)
        g.wait_ge(dsem, 64 + 96 * nchunk + 16)
    return nc


def _build_inputs(x, W_ih, W_hh, b_ih, b_hh, T_, R_):
    wx = np.empty((2, 256), np.float32)
    wx[0] = b_ih + b_hh
    wx[1] = W_ih[:, 0]
    wh = np.ascontiguousarray(W_hh.T, dtype=np.float32)  # [64, 256]
    in_maps = []
    for c in range(NCORE):
        xp = np.zeros((2, T_ + R_), np.float32)
        xp[:, :T_] = x[2 * c : 2 * c + 2, :, 0]
        in_maps.append({"x": xp, "wx": wx, "wh": wh})
    return in_maps


R_CHUNK = 512
DRAINS = False
_NC_CACHE = {}


def _get_nc(T_, R_, drains):
    key = (T_, R_, drains)
    if key not in _NC_CACHE:
        _NC_CACHE[key] = build_nc(T_, R_, drains=drains)
    return _NC_CACHE[key]


def kernel(x, W_ih, W_hh, b_ih, b_hh):
    x = np.asarray(x, np.float32)
    W_ih = np.asarray(W_ih, np.float32)
    W_hh = np.asarray(W_hh, np.float32)
    b_ih = np.asarray(b_ih, np.float32)
    b_hh = np.asarray(b_hh, np.float32)

    R_ = R_CHUNK
    nc = _get_nc(T, R_, DRAINS)
    in_maps = _build_inputs(x, W_ih, W_hh, b_ih, b_hh, T, R_)
    # A previously wedged device can fail the first run and recover on the
    # next; retry a couple of times before giving up.
    last_err = None
    for _attempt in range(3):
        try:
            res = run_bass_kernel_spmd(nc, in_maps, core_ids=list(range(NCORE)))
            break
        except Exception as e:  # noqa: BLE001
            last_err = e
    else:
        raise last_err

    y = np.empty((B, T, H), np.float32)
    c_out = np.empty((B, T, H), np.float32)
    ap_final = np.empty((B, 2 * H), np.float32)
    for c in range(NCORE):
        r = res.results[c]
        for b in range(2):
            y[2 * c + b] = r["yh"][b].T
            c_out[2 * c + b] = r["cs"][b].T
            ap_final[2 * c + b] = r["apf"][:, b]
    return y, c_out, ap_final
